# revision 63
# baseline (speedup 1.0000x reference)
"""GraphTransformerLayer kernel for 8 Trainium2 NeuronCores.

Sharding: graphs sorted by size into 4 bands of 8; core c takes the c-th
graph of each band (slot). Slot k is padded to the band max (rounded to 4),
so all cores run one identical SPMD program with near-zero padding waste.
Cores are fully independent (no collectives).

Layout is feature-major [dim, node] for q/k/cT/out so matmuls need no
transposes; v is node-major per 128-row key block. Per-core validity is
pure data (zero-padded x, a 0/1 vmask multiplied into v), so the exp needs
no mask bias and fuses across key blocks when 2*S <= 512.

Numerics: b_k is dropped (softmax-invariant), b_v is folded into b_o
(softmax weights sum to 1). All matmul inputs bf16, fp32 PSUM; softmax
denominators via a ones-column in v; reciprocal broadcast to 64 rows with
a K=2 matmul per head pair.

Projections run as fp8(e4m3) hi+lo residual DoubleRow matmuls
(x@W ~ xh@Wh + xh@Wl + xl@Wh at 0.5 cycles/row, pre-scaled by 32/512 to
keep residuals out of the fp8 denormal range) which beats bf16 in both
speed and accuracy; the out-projection stays bf16 (its input is produced
on device). Engine balance: PE matmuls; ACT exp + rb/o-bias copies; DVE
q/k/v copies, reciprocals, normalize multiplies (GPSIMD cannot access
PSUM). Projection and out-projection chunks are interleaved into the
attention step stream with adaptive draining so the PE stays fed while
the ACT exp chain runs.
"""

import os
import sys
from collections import deque

import numpy as np
import ml_dtypes

for _p in ("/opt/trn_rl_repo", "/root/.axon_site/_ro/trn_rl_repo"):
    if os.path.isdir(_p) and _p not in sys.path:
        sys.path.insert(0, _p)

DIM = 512
H = 8
DH = 64
NUM_GRAPHS = 32
N_CORES = 8
NS = NUM_GRAPHS // N_CORES  # slots (graphs) per core
VC = H * (DH + 1)  # 520: v columns (per head: 64 dims + 1 ones col)
SCALE = 1.0 / np.sqrt(DH)
SX, SW = 32.0, 512.0  # fp8 pre-scales (power of two) for x and w_{q,k,v}
ISCALE = 1.0 / (SX * SW)
SQK = 16.0  # fp8 pre-scale for on-device q/k (scores computed x256)

_NC_CACHE = {}
LAST_RESULTS = None



def _build(S):
    import concourse.bass as bass
    import concourse.tile as tile
    from concourse import mybir
    from contextlib import ExitStack

    f32 = mybir.dt.float32
    b16 = mybir.dt.bfloat16
    AF = mybir.ActivationFunctionType

    S = [int(s) for s in S]
    JT = [-(-s // 128) for s in S]
    JSPAN = [jt * 128 for jt in JT]
    O = np.concatenate([[0], np.cumsum(JSPAN)]).astype(int)  # key layout
    P = np.concatenate([[0], np.cumsum(S)]).astype(int)  # query layout
    NPX, NPAD = int(O[NS]), int(P[NS])
    TOTJB = sum(JT)
    jbcol = {}  # (g, jb) -> column in vmask
    for g in range(NS):
        for jb in range(JT[g]):
            jbcol[(g, jb)] = len(jbcol)
    # j-block grouping per slot for score psum tiles / fused exp
    groups = {}
    for g in range(NS):
        if S[g] * JT[g] <= 512:
            groups[g] = [list(range(JT[g]))]
        else:
            groups[g] = [[jb] for jb in range(JT[g])]

    f8 = mybir.dt.float8e4
    nc = bass.Bass()
    # x / w_{q,k,v} as fp8 hi+lo residual pairs in DoubleRow pair layout:
    # tensor[t][p, i, n] = src[256*t + 128*i + p, n] for kb-pair t, slice i
    x8_d = {}
    w8_d = {}
    for hl in ("h", "l"):
        for t in range(2):
            x8_d[(hl, t)] = nc.declare_dram_parameter(
                f"x{hl}{t}", [128, 2, NPX], f8, isOutput=False)
            for w in ("q", "k", "v"):
                cols = VC if w == "v" else DIM
                w8_d[(w, hl, t)] = nc.declare_dram_parameter(
                    f"w{w}{hl}{t}", [128, 2, cols], f8, isOutput=False)
    wo_d = nc.declare_dram_parameter("woT", [DIM, DIM], b16, isOutput=False)
    bq_d = nc.declare_dram_parameter("bq", [128, 4], f32, isOutput=False)
    bo_d = nc.declare_dram_parameter("bo2", [128, 4], f32, isOutput=False)
    v8_d = nc.declare_dram_parameter("vones", [128, 8 * TOTJB], b16, isOutput=False)
    id_d = nc.declare_dram_parameter("ident", [128, 128], b16, isOutput=False)
    out_d = nc.declare_dram_parameter("outT", [DIM, NPAD], b16, isOutput=True)

    with ExitStack() as ctx:
        tc = ctx.enter_context(tile.TileContext(nc))
        wpool = ctx.enter_context(tc.tile_pool(name="w", bufs=1))
        xpool = ctx.enter_context(tc.tile_pool(name="x", bufs=1))
        apool = ctx.enter_context(tc.tile_pool(name="acts", bufs=1))
        vpool = ctx.enter_context(tc.tile_pool(name="v", bufs=1))
        epool = ctx.enter_context(tc.tile_pool(name="e", bufs=12))
        rcpool = ctx.enter_context(tc.tile_pool(name="rc", bufs=6))
        cupool = ctx.enter_context(tc.tile_pool(name="cu", bufs=4))
        cnpool = ctx.enter_context(tc.tile_pool(name="cn", bufs=7))
        ctpool = ctx.enter_context(tc.tile_pool(name="ct", bufs=7))
        opool = ctx.enter_context(tc.tile_pool(name="o", bufs=4))
        ps = ctx.enter_context(tc.tile_pool(
            name="ps", bufs=int(os.environ.get("KPSB", "3")), space="PSUM"))
        sc = ctx.enter_context(tc.tile_pool(
            name="sc", bufs=int(os.environ.get("KSCB", "4")), space="PSUM"))
        avp = ctx.enter_context(tc.tile_pool(
            name="avp", bufs=int(os.environ.get("KAVB", "1")), space="PSUM"))

        # ---- loads (ordered: first-needed first; k proj starts per-kb)
        # loads spread over the HWDGE queues (SP: x, wv, v8; ACT: wk, wq,
        # wo, biases) so the prologue isn't serialized on one queue
        x_sb, w_sb, wo_sb = {}, {}, []
        for hl in ("h", "l"):
            for t in range(2):
                tl = xpool.tile([128, 2, NPX], f8, tag=f"x{hl}{t}",
                                name=f"x{hl}{t}")
                nc.sync.dma_start(tl[:, :, :], x8_d[(hl, t)][:, :, :])
                x_sb[(hl, t)] = tl
                tl = wpool.tile([128, 2, DIM], f8, tag=f"wk{hl}{t}",
                                name=f"wk{hl}{t}")
                nc.scalar.dma_start(tl[:, :, :], w8_d[("k", hl, t)][:, :, :])
                w_sb[("k", hl, t)] = tl
        for t in range(2):
            for hl in ("h", "l"):
                tl = wpool.tile([128, 2, DIM], f8, tag=f"wq{hl}{t}",
                                name=f"wq{hl}{t}")
                nc.scalar.dma_start(tl[:, :, :], w8_d[("q", hl, t)][:, :, :])
                w_sb[("q", hl, t)] = tl
        for t in range(2):
            for hl in ("h", "l"):
                tl = wpool.tile([128, 2, VC], f8, tag=f"wv{hl}{t}",
                                name=f"wv{hl}{t}")
                nc.sync.dma_start(tl[:, :, :], w8_d[("v", hl, t)][:, :, :])
                w_sb[("v", hl, t)] = tl
        bq_sb = wpool.tile([128, 4], f32, tag="bq")
        nc.scalar.dma_start(bq_sb[:], bq_d[:])
        v8_sb = wpool.tile([128, 8 * TOTJB], b16, tag="v8")
        nc.sync.dma_start(v8_sb[:], v8_d[:])
        for kb in range(4):
            t = wpool.tile([128, DIM], b16, tag=f"wo{kb}", name=f"wo{kb}")
            nc.scalar.dma_start(t[:], wo_d[kb * 128:(kb + 1) * 128, :])
            wo_sb.append(t)
        bo_sb = wpool.tile([128, 4], f32, tag="bo")
        nc.scalar.dma_start(bo_sb[:], bo_d[:])
        id_sb = wpool.tile([128, 128], b16, tag="ident")
        nc.scalar.dma_start(id_sb[:], id_d[:])

        # keep the PE continuously busy through the DMA prologue so the
        # p-state is ramped when the projection burst hits the critical path
        wu = wpool.tile([128, 512], b16, tag="wu")
        nc.vector.memset(wu[:], 0.0)
        wup = ps.tile([128, 512], f32, tag="ps", name="wup")
        wuw = int(os.environ.get("KWUW", "512"))
        for _ in range(int(os.environ.get("KWARM", "15"))):
            nc.tensor.matmul(wup[:, :wuw], wu[:, :128], wu[:, :wuw],
                             start=True, stop=True)


        # persistent activations: q/k as fp8 (x16) in DoubleRow pair layout
        # [128, 2, N]; plane i=1 of k is ZERO so a K=128 DR score matmul
        # contracts (64 real dims + 64 zeros) at 0.5 cycles/col. q's i=1
        # plane is garbage (multiplied by the zero plane).
        qT_sb = [apool.tile([128, 2, NPAD], f8, tag=f"q{fb}", name=f"qT{fb}")
                 for fb in range(4)]
        kT_sb = [apool.tile([128, 2, NPX], f8, tag=f"k{fb}", name=f"kT{fb}")
                 for fb in range(4)]
        # plane/pad memsets on Pool; nothing else runs on Pool until the
        # first scores (~12us), so these fill otherwise-idle prologue time
        for fb in range(4):
            nc.gpsimd.memset(kT_sb[fb][:, 1, :], 0.0)
            # q's i=1 plane multiplies k's zero plane, but stale SBUF can
            # hold fp8-NaN bit patterns and NaN*0 = NaN — zero it too
            nc.gpsimd.memset(qT_sb[fb][:, 1, :], 0.0)
            for g in range(NS):
                if JSPAN[g] > S[g]:
                    nc.gpsimd.memset(
                        kT_sb[fb][:, 0, O[g] + S[g]:O[g] + JSPAN[g]], 0.0)
        v_sb = {}
        for g in range(NS):
            for jb in range(JT[g]):
                v_sb[(g, jb)] = vpool.tile([128, VC], b16, tag=f"v{g}_{jb}",
                                           name=f"v{g}_{jb}")

        # ---- projection chunks: fp8 hi/lo residual DoubleRow matmuls.
        # x@W ~ xh@Wh + xh@Wl + xl@Wh, each pass 2 DR matmuls (K=256),
        # psum scaled back by 1/(SX*SW) during the copy out.
        DR = mybir.MatmulPerfMode.DoubleRow
        PASSES = (("h", "h"), ("h", "l"), ("l", "h"))

        def dr_proj(p, w, wcols, xcols, width):
            n = 0
            for (xhl, whl) in PASSES:
                for t in range(2):
                    n += 1
                    nc.tensor.matmul(
                        p[:, :width],
                        w_sb[(w, whl, t)][:, :, wcols],
                        x_sb[(xhl, t)][:, :, xcols],
                        start=(n == 1), stop=(n == 6),
                        perf_mode=DR,
                    )

        def k_chunk(g, fb, pool=None, ptag="ps"):
            def emit():
                pl = pool or ps
                span = S[g]
                p = pl.tile([128, 512], f32, tag=ptag, name="psk")
                dr_proj(p, "k", slice(fb * 128, (fb + 1) * 128),
                        slice(O[g], O[g] + span), span)
                nc.vector.tensor_scalar_mul(
                    kT_sb[fb][:, 0, O[g]:O[g] + span], p[:, :span],
                    ISCALE * SQK)
            return emit

        def q_chunk(g, fb, pool=None, ptag="ps"):
            def emit():
                pl = pool or ps
                s = S[g]
                p = pl.tile([128, 512], f32, tag=ptag, name="psq")
                dr_proj(p, "q", slice(fb * 128, (fb + 1) * 128),
                        slice(O[g], O[g] + s), s)
                nc.vector.tensor_scalar(
                    qT_sb[fb][:, 0, P[g]:P[g] + s], p[:, :s], ISCALE * SQK,
                    bq_sb[:, fb:fb + 1], mybir.AluOpType.mult,
                    mybir.AluOpType.add)
            return emit

        def v_chunk(g, jb, pool=None, ptag="ps"):
            def emit():
                pl = pool or ps
                vt = v_sb[(g, jb)]
                col0 = O[g] + jb * 128
                mcol = jbcol[(g, jb)]
                for (off, w) in ((0, 512), (512, VC - 512)):
                    p = pl.tile([128, 512], f32, tag=ptag, name="psv")
                    n = 0
                    for (xhl, whl) in PASSES:
                        for t in range(2):
                            n += 1
                            nc.tensor.matmul(
                                p[:, :w],
                                x_sb[(xhl, t)][:, :, col0:col0 + 128],
                                w_sb[("v", whl, t)][:, :, off:off + w],
                                start=(n == 1), stop=(n == 6),
                                perf_mode=DR,
                            )
                    nc.vector.tensor_scalar_mul(vt[:, off:off + w], p[:, :w],
                                                ISCALE)
                # ones columns (validity) overwrite the zero wv columns
                # (DVE, not Pool: Pool runs the big plane memsets early and
                # these must not queue behind them)
                nc.vector.tensor_copy(
                    vt[:, DH::DH + 1], v8_sb[:, mcol * 8:(mcol + 1) * 8])
            return emit

        o_ps = {}

        def o_piece(g, fb, qb, cts):
            def emit():
                s, jt = S[g], JT[g]
                if qb == 0:
                    o_ps[(g, fb)] = ps.tile([128, 512], f32, tag="ps",
                                            name="pso")
                p = o_ps[(g, fb)]
                qw = min(128, s - 128 * qb)
                for kb in range(4):
                    nc.tensor.matmul(
                        p[:, 128 * qb:128 * qb + qw],
                        wo_sb[kb][:, fb * 128:(fb + 1) * 128],
                        cts[qb][:, kb, :qw],
                        start=(kb == 0), stop=(kb == 3),
                    )
                if g == NS - 1:
                    # tail graph: bias+DMA per qb so the final DMA is small
                    # and earlier pieces stream out while later ones compute
                    ot = opool.tile([128, 128], b16, tag="ot2", name="ot2")
                    nc.vector.tensor_scalar_add(
                        ot[:, :qw], p[:, 128 * qb:128 * qb + qw],
                        bo_sb[:, fb:fb + 1])
                    dma_eng = nc.sync if (fb + qb) % 2 == 0 else nc.scalar
                    dma_eng.dma_start(
                        out_d[fb * 128:(fb + 1) * 128,
                              P[g] + 128 * qb:P[g] + 128 * qb + qw],
                        ot[:, :qw])
                    if qb == jt - 1:
                        del o_ps[(g, fb)]
                elif qb == jt - 1:
                    del o_ps[(g, fb)]
                    ot = opool.tile([128, 512], b16, tag="ot", name="ot")
                    nc.scalar.activation(ot[:, :s], p[:, :s], AF.Identity,
                                         bias=bo_sb[:, fb:fb + 1])
                    dma_eng = nc.sync if fb % 2 == 0 else nc.scalar
                    dma_eng.dma_start(
                        out_d[fb * 128:(fb + 1) * 128, P[g]:P[g] + s],
                        ot[:, :s])
            return emit

        def proj_chunks(g, pool=None, ptag="ps"):
            out = []
            for fb in range(4):
                out.append(("kq", k_chunk(g, fb, pool, ptag)))
            for fb in range(4):
                out.append(("kq", q_chunk(g, fb, pool, ptag)))
            for jb in range(JT[g]):
                out.append(("v", v_chunk(g, jb, pool, ptag)))
            return out

        # ---- attention step pieces
        def emit_scores(g, h):
            s = S[g]
            fb, po = h // 2, 64 * (h % 2)
            ets = [None] * JT[g]
            for grp in groups[g]:
                sct = sc.tile([128, 512], f32, tag="sc", name="sct")
                for i, jb in enumerate(grp):
                    jcol = O[g] + jb * 128
                    nc.tensor.matmul(
                        sct[:, i * s:(i + 1) * s],
                        kT_sb[fb][po:po + 64, :, jcol:jcol + 128],
                        qT_sb[fb][po:po + 64, :, P[g]:P[g] + s],
                        start=True, stop=True,
                        perf_mode=DR,
                        tile_position=(po, 0),
                    )
                et = epool.tile([128, 512], b16, tag="et", name="et")
                w = len(grp) * s
                nc.scalar.activation(et[:, :w], sct[:, :w], AF.Exp,
                                     scale=float(SCALE / (SQK * SQK)))
                for i, jb in enumerate(grp):
                    ets[jb] = et[:, i * s:(i + 1) * s]
            return ets

        def emit_attnv(g, h, ets, avt):
            s, jt = S[g], JT[g]
            hh = h % 2
            for qb in range(jt):
                qw = min(128, s - 128 * qb)
                for jb in range(jt):
                    nc.tensor.matmul(
                        avt[:qw, qb, hh, :],
                        ets[jb][:, 128 * qb:128 * qb + qw],
                        v_sb[(g, jb)][:, 65 * h:65 * h + 65],
                        start=(jb == 0), stop=(jb == jt - 1),
                    )

        def emit_post(g, pair, avt, cns):
            # pair of heads done: psum->sbuf copy, batched reciprocal, then
            # per-(qb) normalize on the (otherwise idle) Pool engine into the
            # node-major c tiles.
            jt = JT[g]
            cu = cupool.tile([128, jt, 2, 65], f32, tag="cu", name="cu")
            nc.vector.tensor_copy(cu[:, :, :, :], avt[:, :, :, :])
            rc = rcpool.tile([128, jt, 2], f32, tag="rc", name="rc")
            nc.vector.reciprocal(rc[:, :, :], cu[:, :, :, 64])
            if GPOS[g] < PE_TR and NORM_ENG == "pool":
                # Pool re-copies rc itself: the copy's plain AP carries the
                # DVE->Pool dependency; the broadcast reads below then only
                # depend on Pool's own in-order stream. (A broadcast_to in1
                # read does not sync correctly against a cross-engine write.)
                rcp = rcpool.tile([128, jt, 2], f32, tag="rcp", name="rcp")
                nc.gpsimd.tensor_copy(rcp[:, :, :], rc[:, :, :])
            for qb in range(jt):
                if GPOS[g] >= PE_TR or NORM_ENG == "dve":
                    for hh in range(2):
                        nc.vector.tensor_scalar_mul(
                            cns[qb][:, 2 * pair + hh, :],
                            cu[:, qb, hh, 0:64],
                            rc[:, qb, hh:hh + 1])
                elif NORM_ENG == "pool2":
                    # per-partition-scalar multiply on Pool (plain APs; the
                    # broadcast_to tensor_mul variant miscomputes on HW)
                    for hh in range(2):
                        nc.gpsimd.tensor_scalar_mul(
                            cns[qb][:, 2 * pair + hh, :],
                            cu[:, qb, hh, 0:64],
                            rc[:, qb, hh:hh + 1])
                else:
                    nc.gpsimd.tensor_mul(
                        cns[qb][:, 2 * pair:2 * pair + 2, :],
                        cu[:, qb, :, 0:64],
                        rcp[:, qb, :, None].broadcast_to([128, 2, 64]),
                    )

        # ---- global schedule: one continuous (g, h) step stream.
        # fill holds (graph_id, chunk); proj(g) is force-drained before
        # ATT(g) starts; otherwise drained adaptively so PE fill work is
        # spread over the whole attention stream.
        fill = deque()  # ((gid, kind), not_before_t, emit_fn)
        t_now = [0]

        def force_proj(g, kinds=("kq", "v")):
            rest = deque()
            while fill:
                key, nb, fn = fill.popleft()
                if key[0] == g and key[1] in kinds:
                    fn()
                else:
                    rest.append((key, nb, fn))
            fill.extend(rest)

        DRAIN_WIN = int(os.environ.get("KDRAIN", "0"))  # 0 = even spread

        def drain_adaptive(iters_left):
            ready = sum(1 for _, nb, _ in fill if nb <= t_now[0])
            if DRAIN_WIN:
                iters_left = min(iters_left, DRAIN_WIN)
            k = -(-len(fill) // max(1, iters_left))
            for _ in range(min(k, ready)):
                if fill[0][1] > t_now[0]:
                    break
                fill.popleft()[2]()

        p0 = (proj_chunks(0, sc, "sc"), proj_chunks(0))
        for i in range(len(p0[0])):
            p0[i % 2][i][1]()
        for g in range(1, NS):
            fill.extend(((g, kind), 0, ch) for kind, ch in proj_chunks(g))

        NORM_ENG = os.environ.get("KNORM", "dve")
        D_AV = int(os.environ.get("KD_AV", "4"))
        D_TAIL = int(os.environ.get("KD_TAIL", "2"))
        O_NB = int(os.environ.get("KO_NB", "3"))
        PE_TR = int(os.environ.get("KPE_TR", str(NS - 2)))
        ITERS = NS * H + D_AV  # one step per (g, h) plus pipeline tail
        pend = {}
        PERM = [int(x) for x in
                os.environ.get("KPERM", ",".join(
                    str(i) for i in range(NS))).split(",")]
        GPOS = {g: i for i, g in enumerate(PERM)}
        steps = [(g, h) for g in PERM for h in range(H)]
        ntr = 0
        next_av = [0]

        def dav(tt):
            return D_AV if GPOS[steps[tt][0]] < NS - 1 else D_TAIL

        for t in range(ITERS):
            t_now[0] = t
            while (next_av[0] < len(steps)
                   and next_av[0] + dav(next_av[0]) <= t):
                tt = next_av[0]
                next_av[0] += 1
                g, h = steps[tt]
                pair, jt = h // 2, JT[g]
                if h == 0:
                    pend[("cn", g)] = [
                        cnpool.tile([128, H, DH], b16, tag="cn",
                                    name=f"cn{g}_{qb}") for qb in range(jt)]
                if h % 2 == 0:
                    pend[("avt", g, pair)] = avp.tile(
                        [128, jt, 2, 65], f32, tag="av", name="avt")
                avt = pend[("avt", g, pair)]
                emit_attnv(g, h, pend.pop((g, h)), avt)
                if h % 2 == 1:
                    emit_post(g, pair, pend.pop(("avt", g, pair)),
                              pend[("cn", g)])
                if h == H - 1:
                    cns = pend.pop(("cn", g))
                    cts = []
                    for qb in range(jt):
                        ct = ctpool.tile([128, 4, 128], b16, tag="ct",
                                         name=f"ct{g}_{qb}")
                        if GPOS[g] >= PE_TR:
                            # low-latency PE transpose for the tail graphs
                            tp = avp.tile([128, 4, 128], b16, tag="av",
                                          name="tp")
                            for kb in range(4):
                                nc.tensor.transpose(
                                    tp[:, kb, :],
                                    cns[qb][:, 2 * kb:2 * kb + 2, :],
                                    id_sb[:, :])
                            nc.vector.tensor_copy(ct[:, :, :], tp[:, :, :])
                        else:
                            eng = nc.sync if ntr % 2 == 0 else nc.scalar
                            ntr += 1
                            eng.dma_start_transpose(ct[:, :, :],
                                                    cns[qb][:, :, :])
                        cts.append(ct)
                    fill.extend((("o", "o"), t + O_NB, o_piece(g, fb, qb, cts))
                                for fb in range(4) for qb in range(jt))
            if t < len(steps):
                g, h = steps[t]
                if h == 0:
                    force_proj(g)
                pend[(g, h)] = emit_scores(g, h)
            drain_adaptive(ITERS - t)
        t_now[0] = ITERS
        while fill:
            fill.popleft()[2]()

    _split_multiwaits(nc, mybir)
    return nc, dict(S=S, JT=JT, O=O, P=P, NPX=NPX, NPAD=NPAD, jbcol=jbcol)


def _split_multiwaits(nc, mybir, max_waits=1):
    """The pinned walrus codegen accepts only one sync-wait per instruction;
    move extra waits onto dedicated NoOps just before the instruction (same
    engine stream, so semantics are identical)."""
    n_split = 0
    for fn in nc.m.functions:
        for blk in fn.blocks:
            new_insts = []
            for inst in blk.instructions:
                si = getattr(inst, "sync_info", None)
                if si is not None and si.on_wait and len(si.on_wait) > max_waits:
                    waits = list(si.on_wait)
                    extra, keep = waits[:-max_waits], waits[-max_waits:]
                    for i, w in enumerate(extra):
                        new_insts.append(mybir.InstNoOp(
                            name=f"{inst.name}-w{i}",
                            sync_info=mybir.SyncInfo(on_wait=[w], on_update=[]),
                            engine=inst.engine,
                            bass_nofuse=True,
                        ))
                    inst.sync_info = mybir.SyncInfo(on_wait=keep,
                                                    on_update=si.on_update)
                    n_split += 1
                new_insts.append(inst)
            blk.instructions = new_insts
    return n_split


def _get_nc(S):
    key = tuple(S)
    if key not in _NC_CACHE:
        _NC_CACHE[key] = _build(key)
    return _NC_CACHE[key]


def _plan(counts):
    order = np.argsort(-counts, kind="stable")
    S = []
    for k in range(NS):
        band = counts[order[k * N_CORES:(k + 1) * N_CORES]]
        S.append(max(8, -(-int(band.max()) // 4) * 4))
    return order, tuple(S)


def kernel(x, batch, w_q, w_k, w_v, b_q, b_k, b_v, w_o, b_o):
    global LAST_RESULTS
    x = np.asarray(x, np.float32)
    batch = np.asarray(batch)
    counts = np.bincount(batch, minlength=NUM_GRAPHS)[:NUM_GRAPHS]
    starts = np.concatenate([[0], np.cumsum(counts)]).astype(np.int64)
    order, S = _plan(counts)
    assert max(S) <= 512, f"graph too large: {counts.max()}"
    nc, meta = _get_nc(S)
    JT, O, P = meta["JT"], meta["O"], meta["P"]
    NPX, NPAD, jbcol = meta["NPX"], meta["NPAD"], meta["jbcol"]
    TOTJB = len(jbcol)

    bf16 = ml_dtypes.bfloat16
    fp8 = ml_dtypes.float8_e4m3

    def pair_hilo(mT):
        # mT [512, cols] fp32 (pre-scaled) -> {('h'|'l', t): [128, 2, cols]}
        hi = mT.astype(fp8)
        lo = (mT - hi.astype(np.float32)).astype(fp8)
        out = {}
        for hl, m in (("h", hi), ("l", lo)):
            for t in range(2):
                out[(hl, t)] = np.ascontiguousarray(
                    m[256 * t:256 * (t + 1)].reshape(2, 128, -1)
                    .transpose(1, 0, 2))
        return out

    woT = np.ascontiguousarray(w_o.T).astype(bf16)
    wq8 = pair_hilo(np.ascontiguousarray(w_q.T) * SW)
    wk8 = pair_hilo(np.ascontiguousarray(w_k.T) * SW)
    wvT = np.zeros((DIM, VC), np.float32)
    for h in range(H):
        wvT[:, 65 * h:65 * h + 64] = w_v[64 * h:64 * h + 64, :].T
    wv8 = pair_hilo(wvT * SW)
    bq = np.ascontiguousarray(b_q.reshape(4, 128).T.astype(np.float32)) * SQK
    bo2v = (b_o + w_o @ b_v).astype(np.float32)
    bo2 = np.ascontiguousarray(bo2v.reshape(4, 128).T)

    in_maps = []
    for c in range(N_CORES):
        xs = np.zeros((NPX, DIM), np.float32)
        vmask = np.zeros((128, TOTJB), np.float32)
        for k in range(NS):
            g = order[k * N_CORES + c]
            n = int(counts[g])
            xs[O[k]:O[k] + n] = x[starts[g]:starts[g] + n]
            for jb in range(JT[k]):
                nvalid = min(128, max(0, n - jb * 128))
                vmask[:nvalid, jbcol[(k, jb)]] = 1.0
        vones = np.repeat(vmask, 8, axis=1)
        x8 = pair_hilo(np.ascontiguousarray(xs.T) * SX)
        im = {"woT": woT, "bq": bq, "bo2": bo2,
              "vones": np.ascontiguousarray(vones).astype(bf16),
              "ident": np.eye(128, dtype=bf16)}
        for (hl, t), v8 in x8.items():
            im[f"x{hl}{t}"] = v8
        for w, w8 in (("q", wq8), ("k", wk8), ("v", wv8)):
            for (hl, t), v8 in w8.items():
                im[f"w{w}{hl}{t}"] = v8
        in_maps.append(im)

    from concourse.bass_utils import run_bass_kernel_spmd
    trace = os.environ.get("KTRACE", "") not in ("", "0")
    LAST_RESULTS = run_bass_kernel_spmd(nc, in_maps, list(range(N_CORES)),
                                        trace=trace)

    out = np.empty((x.shape[0], DIM), np.float32)
    for c in range(N_CORES):
        oT = LAST_RESULTS.results[c]["outT"]
        for k in range(NS):
            g = order[k * N_CORES + c]
            n = int(counts[g])
            out[starts[g]:starts[g] + n] = oT[:, P[k]:P[k] + n].T.astype(np.float32)
    return out

